# revision 20
# baseline (speedup 1.0000x reference)
# Bass/Trainium2 kernel for nn_Net_57423712747598 (Mamba-based pansharpening).
# Host-side numpy preprocessing + one SPMD NEFF on 8 NeuronCores.
import numpy as np
import ml_dtypes

import concourse.bass as bass
import concourse.tile as tile
import concourse.tile_utils as tile_utils
from concourse import bacc, mybir
from concourse.bass_utils import run_bass_kernel_spmd
from concourse.masks import make_identity

tile_utils.max_sbuf_usage = 204 * 1024

F32 = mybir.dt.float32
BF16 = mybir.dt.bfloat16
AF = mybir.ActivationFunctionType
ALU = mybir.AluOpType
AX = mybir.AxisListType
NPBF = ml_dtypes.bfloat16

L = 4096
TCH = 512     # front-end chunk (one PSUM bank)
NT = L // TCH
SCH = 1024    # scan-section chunk
NS = L // SCH
NJ = 8        # scan tiles (n pairs)
D_CONV = 4


def _np(x):
    return np.asarray(x, dtype=np.float32)


def _bf(x):
    return np.asarray(x).astype(NPBF)


# ============================================================================
# Host parameter packing
# ============================================================================

def _conv_pack(w, b, scale=1.0):
    """w: (O, I, 3, 3) -> [96, 3, O] rows=(dx, c) per dy; b: (O,)"""
    w = _np(w) * scale
    O, I, _, _ = w.shape
    out = np.zeros((96, 3, O), np.float32)
    for dy in range(3):
        for s in range(3):
            out[s * 32:s * 32 + I, dy, :] = w[:, :, dy, s].transpose(1, 0)
    return _bf(out), _np(b).reshape(-1, 1)


def _dw_dense(w):
    w = _np(w)
    C = w.shape[0]
    dense = np.zeros((C, C, 3, 3), np.float32)
    for c in range(C):
        dense[c, c] = w[c, 0]
    return dense


def _dwconv1d_pack(w, b):
    """(64,1,4) -> lhsT [64, 4, 128] doubled (bf16); bias [1, 128] f32"""
    w = _np(w)
    out = np.zeros((64, D_CONV, 128), np.float32)
    for k in range(D_CONV):
        for d in range(64):
            out[d, k, d] = w[d, 0, k]
            out[d, k, d + 64] = w[d, 0, k]
    b = _np(b)
    return _bf(out), np.concatenate([b, b]).reshape(1, 128)


def _ln_fold(nw, nb, win):
    nw, nb, win = _np(nw), _np(nb), _np(win)
    return nw[:, None] * win, (nb @ win).reshape(-1, 1)


def _mamba_pack(m, pref, w):
    A = -np.exp(_np(m['Alog']))  # (64, 16)
    acol = np.zeros((128, NJ), np.float32)
    for j in range(NJ):
        acol[0:64, j] = A[:, 2 * j]
        acol[64:128, j] = A[:, 2 * j + 1]
    w[pref + 'acol'] = acol
    w[pref + 'dwl'], w[pref + 'dwb'] = _dwconv1d_pack(m['convw'], m['convb'])
    wx = _np(m['Wx']) * 0.5                       # u2 fold
    wxd = np.zeros((64, 66), np.float32)
    wxd[:, 0:16] = wx[:, 2:18]      # B
    wxd[:, 32:48] = wx[:, 18:34]    # C
    wxd[:, 64:66] = wx[:, 0:2]      # dt-rank
    w[pref + 'wx'] = _bf(wxd)
    wdt = _np(m['Wdt'])
    w[pref + 'wdt2'] = np.concatenate([wdt, wdt], axis=1)   # (2, 128)
    bdt = _np(m['bdt'])
    w[pref + 'bdt2'] = np.concatenate([bdt, bdt]).reshape(128, 1)
    w[pref + 'dphalf'] = (_np(m['Dp']) * 0.5).reshape(64, 1)
    w[pref + 'wout'] = _bf(_np(m['Wout']) * 0.5)  # sz2 fold


def prep_inputs(hs, pan, params):
    w = {}

    def enc(pp, pref, scale0=1.0):
        a, b = _conv_pack(pp['w'], pp['b'], scale=scale0)
        w[pref + 'c0w'], w[pref + 'c0b'] = a, b
        for i, bp in enumerate(pp['blocks']):
            a, b = _conv_pack(bp['c1w'], bp['c1b'])
            w[f'{pref}b{i}c1w'], w[f'{pref}b{i}c1b'] = a, b
            a, b = _conv_pack(bp['c2w'], bp['c2b'])
            w[f'{pref}b{i}c2w'], w[f'{pref}b{i}c2b'] = a, b
            w[f'{pref}b{i}inw'] = _np(bp['inw']).reshape(16, 1)
            w[f'{pref}b{i}inb'] = _np(bp['inb']).reshape(16, 1)

    enc(params['hs_enc1'], 'he1')
    enc(params['pan_enc1'], 'pe1')
    enc(params['hs_enc'], 'hse', scale0=0.5)   # dwt /2 fold
    hw = _np(params['hs_enc']['w'])
    a, _ = _conv_pack(hw[:, 0:31], params['hs_enc']['b'], scale=0.5)
    w['hsec0wa'] = a
    a, _ = _conv_pack(hw[:, 31:32], params['hs_enc']['b'], scale=0.5)
    w['hsec0wb'] = a
    enc(params['pan_enc'], 'pse', scale0=0.5)  # dwt /2 fold
    for nm in ('sf1', 'sf2'):
        sw = _np(params[nm + 'w'])
        ga, _ = _conv_pack(sw[:, 0:32], params[nm + 'b'])
        gb, _ = _conv_pack(sw[:, 32:64], params[nm + 'b'])
        w[nm + 'wa'], w[nm + 'wb'], w[nm + 'bb'] = ga, gb, _np(params[nm + 'b']).reshape(-1, 1)
    w['hstokw'] = _bf(_np(params['hs_tokw'])[:, :, 0, 0].T)
    w['hstokb'] = _np(params['hs_tokb']).reshape(32, 1)
    w['pantokw'] = _bf(_np(params['pan_tokw'])[:, :, 0, 0].T)
    w['pantokb'] = _np(params['pan_tokb']).reshape(32, 1)

    for name, blocks in (('ms', params['ms_fe']), ('pn', params['pan_fe'])):
        for i, bp in enumerate(blocks):
            pref = f'{name}{i}_'
            W1, b1 = _ln_fold(bp['nw'], bp['nb'], bp['m']['Win'])
            w[pref + 'win'] = W1
            w[pref + 'winba'] = b1[0:64]
            w[pref + 'winbb'] = b1[64:128]
            _mamba_pack(bp['m'], pref, w)
    for i, bp in enumerate(params['df']):
        pref = f'df{i}_'
        w[pref + 'n1w'] = np.repeat(_np(bp['n1w']).reshape(1, 32), 128, 0)
        w[pref + 'n1b'] = np.repeat(_np(bp['n1b']).reshape(1, 32), 128, 0)
        W1, b1 = _ln_fold(bp['n1w'], bp['n1b'], bp['m']['Win'])
        w[pref + 'win'] = W1
        w[pref + 'winba'] = b1[0:64]
        w[pref + 'winbb'] = b1[64:128]
        W2, b2 = _ln_fold(bp['n2w'], bp['n2b'], bp['m']['Win2'])
        w[pref + 'win2'], w[pref + 'win2b'] = W2, b2
        _mamba_pack(bp['m'], pref, w)
        w[pref + 'dwl2'], w[pref + 'dwb2'] = _dwconv1d_pack(
            bp['m']['conv2w'], bp['m']['conv2b'])
        a, b = _conv_pack(_dw_dense(bp['dww']), bp['dwb'])
        w[pref + 'dwsw'], w[pref + 'dwsb'] = a, b
    op = params['out']
    for nm in ('ci', 'p1', 'p2', 'cl'):
        a, b = _conv_pack(op[nm + 'w'], op[nm + 'b'])
        w['rf' + nm + 'w'], w['rf' + nm + 'b'] = a, b
    w['rfd1w'] = _np(op['d1w'])[:, :, 0, 0].T.copy()
    w['rfd1b'] = _np(op['d1b']).reshape(8, 1)
    w['rfd2w'] = _np(op['d2w'])[:, :, 0, 0].T.copy()
    w['rfd2bneg'] = -_np(op['d2b']).reshape(32, 1)

    sel = np.zeros((48, NJ, 128), np.float32)
    for j in range(NJ):
        sel[2 * j, j, 0:64] = 1.0
        sel[2 * j + 1, j, 64:128] = 1.0
    sel[32:48] = sel[0:16]
    w['scansel'] = _bf(sel)
    g = np.zeros((128, 64), np.float32)
    g[0:64] = np.eye(64, dtype=np.float32)
    g[64:128] = np.eye(64, dtype=np.float32)
    w['scang'] = _bf(g)
    w['onesrow'] = np.ones((1, TCH), np.float32)

    maps = []
    for b in range(2):
        m = dict(w)
        m['hs'] = _np(hs[b]).reshape(31, 256)
        m['pan'] = _np(pan[b]).reshape(1, 4096)
        maps.append(m)
    return maps


# ============================================================================
# Device program
# ============================================================================

def sview(t, P, S, row0, col0):
    """strided [P, S/2, S/2] view of [P, S*S] map at (row0::2, col0::2)"""
    a = t[:]
    return bass.AP(tensor=a.tensor, offset=a.offset + row0 * S + col0,
                   ap=[[a.ap[0][0], P], [2 * S, S // 2], [2, S // 2]])


class Net:
    def __init__(self, nc, tc, ctx, in_specs):
        self.nc, self.tc = nc, tc
        self.cache = {}
        self.dram = {}
        for name, (shp, npdt) in in_specs.items():
            self.dram[name] = nc.dram_tensor('in_' + name, list(shp),
                                             mybir.dt.from_np(np.dtype(npdt)),
                                             kind="ExternalInput").ap()
        self.out_dram = nc.dram_tensor('out', [31, 64, 64], F32,
                                       kind="ExternalOutput").ap()
        self.consts = ctx.enter_context(tc.tile_pool(name="consts", bufs=1))
        self.wpool = ctx.enter_context(tc.tile_pool(name="wpool", bufs=2))
        self.work1 = ctx.enter_context(tc.tile_pool(name="work1", bufs=1))
        self.work = ctx.enter_context(tc.tile_pool(name="work", bufs=2))
        self.psum = ctx.enter_context(tc.tile_pool(name="psum", bufs=2, space="PSUM"))
        self.dramp = ctx.enter_context(tc.tile_pool(name="dramp", bufs=1, space="DRAM"))
        self.ident = self.consts.tile([128, 128], F32)
        make_identity(nc, self.ident)
        self.onesrow = self.gconst('onesrow')
        self.eps = self.consts.tile([128, 1], F32, tag='c_eps')
        self.nc.vector.memset(self.eps[:], 1e-5)
        self.fm = None

    def _dt(self, name):
        return self.dram[name].dtype

    def gconst(self, name):
        if name not in self.cache:
            ap = self.dram[name]
            t = self.consts.tile(list(ap.shape), ap.dtype, tag='c_' + name)
            self.nc.sync.dma_start(t[:], ap)
            self.cache[name] = t
        return self.cache[name]

    def bconst(self, name, tag):
        ap = self.dram[name]
        t = self.wpool.tile(list(ap.shape), ap.dtype, tag=tag)
        self.nc.sync.dma_start(t[:], ap)
        return t

    # --- generic conv3x3 (SAME, stride 1) over [C<=32, S*S] maps ---
    def conv3x3(self, xs, wnames, bname, S, out_t, act='lrelu', resid=None,
                cout=32):
        nc = self.nc
        HWn = S * S
        bias = self.bconst(bname, 'cwb')
        wts = [self.bconst(wn, f'cw{gi}') for gi, wn in enumerate(wnames)]
        stacks = []
        for gi, x in enumerate(xs):
            st = self.work1.tile([96, HWn], BF16, tag=f'cstack{gi}')
            nc.vector.tensor_copy(st[32:64, :], x[0:32, :])
            nc.scalar.copy(st[0:32, 1:HWn], x[0:32, 0:HWn - 1])
            nc.vector.memset(st[0:32, 0:1], 0.0)
            s0 = st[0:32, :].rearrange("c (r w) -> c r w", r=S)
            nc.vector.memset(s0[:, :, 0:1], 0.0)
            nc.scalar.copy(st[64:96, 0:HWn - 1], x[0:32, 1:HWn])
            s2 = st[64:96, :].rearrange("c (r w) -> c r w", r=S)
            nc.vector.memset(s2[:, :, S - 1:S], 0.0)
            stacks.append(st)
        rpc = 512 // S
        bias_ap = bias[:]
        for r0 in range(0, S, rpc):
            r1 = r0 + rpc
            ps = self.psum.tile([cout, rpc, S], F32, tag='mm')
            first = True
            for gi, st in enumerate(stacks):
                stv = st[:].rearrange("c (r w) -> c r w", r=S)
                for k, dy in enumerate((0, -1, 1)):
                    a0, a1 = max(r0, -dy), min(r1, S - dy)
                    if a0 >= a1:
                        continue
                    last = (gi == len(stacks) - 1) and (k == 2)
                    nc.tensor.matmul(ps[:, a0 - r0:a1 - r0, :],
                                     wts[gi][:, dy + 1, :],
                                     stv[:, a0 + dy:a1 + dy, :],
                                     start=first, stop=last)
                    first = False
            ov = out_t[0:cout, :].rearrange("c (r w) -> c r w", r=S)[:, r0:r1, :]
            if act == 'lrelu':
                nc.scalar.activation(ov, ps[:], AF.Lrelu, bias=bias_ap, alpha=0.2)
            elif act == 'relu':
                nc.scalar.activation(ov, ps[:], AF.Relu, bias=bias_ap)
            else:
                nc.scalar.activation(ov, ps[:], AF.Identity, bias=bias_ap)
            if resid is not None:
                rv = resid[0:cout, :].rearrange("c (r w) -> c r w", r=S)[:, r0:r1, :]
                nc.vector.tensor_tensor(ov, ov, rv, op=ALU.add)
        return out_t

    def instnorm(self, x, wname, bname, HWn):
        nc = self.nc
        xv = x[0:16, :]
        nchunk = HWn // 512
        stats = self.work.tile([16, nchunk, 6], F32, tag='in_st')
        for i in range(nchunk):
            nc.vector.bn_stats(stats[:, i, :], x[0:16, i * 512:(i + 1) * 512])
        mv = self.work.tile([16, 2], F32, tag='in_mv')
        nc.vector.bn_aggr(mv[:], stats[:])
        sd = self.work.tile([16, 1], F32, tag='in_sd')
        nc.scalar.activation(sd[:], mv[:, 1:2], AF.Sqrt, bias=self.eps[0:16, :])
        inv = self.work.tile([16, 1], F32, tag='in_inv')
        nc.vector.reciprocal(inv[:], sd[:])
        g = self.bconst(wname, 'cing')
        b = self.bconst(bname, 'cinb')
        sc = self.work.tile([16, 1], F32, tag='in_sc')
        nc.vector.tensor_tensor(sc[:], g[:], inv[:], op=ALU.mult)
        bb = self.work.tile([16, 1], F32, tag='in_bb')
        nc.vector.tensor_tensor(bb[:], mv[:, 0:1], sc[:], op=ALU.mult)
        nc.vector.tensor_tensor(bb[:], b[:], bb[:], op=ALU.subtract)
        nc.scalar.activation(xv, xv, AF.Identity, scale=sc[:], bias=bb[:])

    def hin(self, x, pref, S, out_tag):
        r = self.fm.tile([32, S * S], BF16, tag='hin_r' if S == 64 else 'hin_r32')
        self.conv3x3([x], [pref + 'c1w'], pref + 'c1b', S, r)
        self.instnorm(r, pref + 'inw', pref + 'inb', S * S)
        out = self.fm.tile([32, S * S], BF16, tag=out_tag)
        self.conv3x3([r], [pref + 'c2w'], pref + 'c2b', S, out, resid=x)
        return out

    def encoder(self, x, pref, S, ta, tb, c0w=None):
        y = self.fm.tile([32, S * S], BF16, tag=ta)
        xs = x if isinstance(x, list) else [x]
        self.conv3x3(xs, c0w or [pref + 'c0w'], pref + 'c0b', S, y, act='none')
        y = self.hin(y, pref + 'b0', S, tb)
        y = self.hin(y, pref + 'b1', S, ta)
        y = self.hin(y, pref + 'b2', S, tb)
        return y

    # --- layernorm in token-transposed form ---
    def ln_T(self, xT, outT, affine=None):
        nc = self.nc
        sq = self.work.tile([128, 32, 32], F32, tag='ln_sq')
        nc.scalar.activation(sq[:], xT[:], AF.Square)
        s1 = self.work.tile([128, 32], F32, tag='ln_s1')
        s2 = self.work.tile([128, 32], F32, tag='ln_s2')
        nc.vector.tensor_reduce(s1[:], xT[:], axis=AX.X, op=ALU.add)
        nc.vector.tensor_reduce(s2[:], sq[:], axis=AX.X, op=ALU.add)
        mu = self.work.tile([128, 32], F32, tag='ln_mu')
        nc.scalar.mul(mu[:], s1[:], 1.0 / 32.0)
        e2 = self.work.tile([128, 32], F32, tag='ln_e2')
        nc.scalar.mul(e2[:], s2[:], 1.0 / 32.0)
        var = self.work.tile([128, 32], F32, tag='ln_var')
        nc.vector.tensor_tensor(var[:], mu[:], mu[:], op=ALU.mult)
        nc.vector.tensor_tensor(var[:], e2[:], var[:], op=ALU.subtract)
        sd = self.work.tile([128, 32], F32, tag='ln_sd')
        nc.scalar.activation(sd[:], var[:], AF.Sqrt, bias=self.eps[:])
        inv = self.work.tile([128, 32], F32, tag='ln_inv')
        nc.vector.reciprocal(inv[:], sd[:])
        muinv = self.work.tile([128, 32], F32, tag='ln_muinv')
        nc.vector.tensor_tensor(muinv[:], mu[:], inv[:], op=ALU.mult)

        def bc(t):
            a = t[:]
            return bass.AP(tensor=a.tensor, offset=a.offset,
                           ap=[a.ap[0], a.ap[1], [0, 32]])

        nc.vector.tensor_tensor(outT[:], xT[:], bc(inv), op=ALU.mult)
        nc.vector.tensor_tensor(outT[:], outT[:], bc(muinv), op=ALU.subtract)
        if affine is not None:
            wt, bt = affine

            def bcw(t):
                a = t[:]
                return bass.AP(tensor=a.tensor, offset=a.offset,
                               ap=[a.ap[0], [0, 32], a.ap[1]])
            nc.vector.tensor_tensor(outT[:], outT[:], bcw(wt), op=ALU.mult)
            nc.vector.tensor_tensor(outT[:], outT[:], bcw(bt), op=ALU.add)

    def transpose_back(self, xT, out):
        """xT [128, 32, 32] -> out [32, L]"""
        nc = self.nc
        for q in range(8):
            ps = self.psum.tile([32, 512], F32, tag='mm')
            for k in range(4):
                jj = q * 4 + k
                nc.tensor.transpose(ps[:, k * 128:(k + 1) * 128], xT[:, jj, :],
                                    self.ident[:])
            nc.scalar.copy(out[:, q * 512:(q + 1) * 512], ps[:])

    def transpose_to(self, x, outT):
        """x [32, L] -> outT [128, 32, 32]"""
        nc = self.nc
        for q in range(8):
            ps = self.psum.tile([128, 4, 32], F32, tag='mm')
            for k in range(4):
                jj = q * 4 + k
                nc.tensor.transpose(ps[:, k, :], x[0:32, jj * 128:(jj + 1) * 128],
                                    self.ident[0:32, 0:32])
            nc.scalar.copy(outT[:, q * 4:(q + 1) * 4, :], ps[:])

    def mm_from_T(self, xT, lhsT, out, bias=None, act=None):
        """out[:, :] = evict(lhsT.T @ transpose_chunks(xT)); out [M, L]"""
        nc = self.nc
        M = out.shape[0]
        for q in range(8):
            ps = self.psum.tile([32, 512], F32, tag='mm')
            for k in range(4):
                jj = q * 4 + k
                nc.tensor.transpose(ps[:, k * 128:(k + 1) * 128], xT[:, jj, :],
                                    self.ident[:])
            xc = self.work.tile([32, 512], F32, tag='fe_xc')
            nc.scalar.copy(xc[:], ps[:])
            ps2 = self.psum.tile([M, 512], F32, tag='mm')
            nc.tensor.matmul(ps2[:], lhsT[:], xc[:])
            ov = out[:, q * 512:(q + 1) * 512]
            if act == 'identity_bias':
                nc.scalar.activation(ov, ps2[:], AF.Identity, bias=bias)
            else:
                nc.scalar.copy(ov, ps2[:])

    # --- mamba tail: from xnT (normalized, transposed) ---
    def mamba(self, pref, xnT, xT_out, cross=None):
        nc = self.nc
        fm, work = self.fm, self.work
        winw = self.bconst(pref + 'win', 'w_win')
        winba = self.bconst(pref + 'winba', 'w_winba')
        winbb = self.bconst(pref + 'winbb', 'w_winbb')
        xc = fm.tile([64, L], BF16, tag='m_xc')
        zt = fm.tile([64, L], BF16, tag='m_z')
        for q in range(8):
            ps = self.psum.tile([32, 512], F32, tag='mm')
            for k in range(4):
                jj = q * 4 + k
                nc.tensor.transpose(ps[:, k * 128:(k + 1) * 128], xnT[:, jj, :],
                                    self.ident[:])
            xnc = work.tile([32, 512], F32, tag='fe_xc')
            nc.scalar.copy(xnc[:], ps[:])
            ps2 = self.psum.tile([128, 512], F32, tag='mm')
            nc.tensor.matmul(ps2[:], winw[:], xnc[:])
            nc.scalar.activation(xc[:, q * 512:(q + 1) * 512], ps2[0:64, :],
                                 AF.Identity, bias=winba[:])
            nc.scalar.activation(zt[:, q * 512:(q + 1) * 512], ps2[64:128, :],
                                 AF.Identity, bias=winbb[:])
        # dwconv1d (doubled out) + tanh -> u2d = 2*silu
        dwl = self.bconst(pref + 'dwl', 'w_dwl')
        dwb = self.bconst(pref + 'dwb', 'w_dwb')
        u2d = fm.tile([128, L], BF16, tag='m_u2d')
        for c in range(NT):
            t0, t1 = c * TCH, (c + 1) * TCH
            ps = self.psum.tile([128, TCH], F32, tag='mm')
            nc.tensor.matmul(ps[:], dwb[:], self.onesrow[:], start=True, stop=False)
            for k in range(D_CONV):
                sh = 3 - k
                a0 = max(t0, sh)
                if a0 >= t1:
                    continue
                nc.tensor.matmul(ps[:, a0 - t0:TCH], dwl[:, k, :],
                                 xc[:, a0 - sh:t1 - sh],
                                 start=False, stop=(k == D_CONV - 1))
            th = work.tile([128, TCH], F32, tag='fe_th')
            nc.scalar.activation(th[:], ps[:], AF.Tanh, scale=0.5)
            nc.vector.scalar_tensor_tensor(u2d[:, t0:t1], th[:], 1.0, ps[:],
                                           op0=ALU.add, op1=ALU.mult)
        # Wx -> dbl rows; B/C + dt path
        wx = self.bconst(pref + 'wx', 'w_wx')
        wdt2 = self.bconst(pref + 'wdt2', 'w_wdt2')
        bdt2 = self.bconst(pref + 'bdt2', 'w_bdt2')
        bc_t = fm.tile([48, L], BF16, tag='m_bc')
        dt2 = fm.tile([128, L], BF16, tag='m_dt2')
        for c in range(NT):
            t0, t1 = c * TCH, (c + 1) * TCH
            ps = self.psum.tile([66, TCH], F32, tag='mm')
            nc.tensor.matmul(ps[:], wx[:], u2d[0:64, t0:t1])
            nc.scalar.copy(bc_t[0:16, t0:t1], ps[0:16, :])
            nc.scalar.copy(bc_t[32:48, t0:t1], ps[32:48, :])
            dtr = work.tile([2, TCH], F32, tag='fe_dtr')
            nc.scalar.copy(dtr[:], ps[64:66, :])
            ps2 = self.psum.tile([128, TCH], F32, tag='mm')
            nc.tensor.matmul(ps2[:], wdt2[:], dtr[:])
            usp = work.tile([128, TCH], F32, tag='fe_th')
            nc.scalar.activation(usp[:], ps2[:], AF.Exp, bias=bdt2[:])
            nc.scalar.activation(dt2[:, t0:t1], usp[:], AF.Ln, bias=1.0)
        # du2 = (dt2 * 0.5) * scan_u
        scan_u = cross['u2d'] if cross else u2d
        du2 = fm.tile([128, L], BF16, tag='m_du2')
        nc.vector.scalar_tensor_tensor(du2[:], dt2[:], 0.5, scan_u[:],
                                       op0=ALU.mult, op1=ALU.mult)
        # scan
        acol = self.bconst(pref + 'acol', 'w_acol')
        dph = self.bconst(pref + 'dphalf', 'w_dph')
        sel = self.gconst('scansel')
        gmat = self.gconst('scang')
        hb = self.work1.tile([128, NJ], F32, tag='sc_hb')
        wout = self.bconst(pref + 'wout', 'w_wout')
        gc = None
        if cross is not None:
            gc = fm.tile([32, L], BF16, tag='m_gc')
        for c in range(NS):
            t0, t1 = c * SCH, (c + 1) * SCH
            yps0 = self.psum.tile([64, 512], F32, tag='yps')
            yps1 = self.psum.tile([64, 512], F32, tag='yps')
            yps = [yps0, yps1]
            for j in range(NJ):
                dA = work.tile([128, SCH], BF16, tag='sc_dA')
                nc.scalar.activation(dA[:], dt2[:, t0:t1], AF.Exp,
                                     scale=acol[:, j:j + 1])
                rep = self.psum.tile([128, SCH], F32, tag='screp')
                nc.tensor.matmul(rep[:, 0:512], sel[0:16, j, :], bc_t[0:16, t0:t0 + 512])
                nc.tensor.matmul(rep[:, 512:SCH], sel[0:16, j, :], bc_t[0:16, t0 + 512:t1])
                dBu = work.tile([128, SCH], BF16, tag='sc_dBu')
                nc.vector.tensor_tensor(dBu[:], du2[:, t0:t1], rep[:], op=ALU.mult)
                h = work.tile([128, SCH], BF16, tag='sc_h')
                init = 0.0 if c == 0 else hb[:, j:j + 1]
                nc.vector.tensor_tensor_scan(h[:], dA[:], dBu[:], init,
                                             op0=ALU.mult, op1=ALU.add)
                if c < NS - 1:
                    nc.vector.tensor_copy(hb[:, j:j + 1], h[:, SCH - 1:SCH])
                rep2 = self.psum.tile([128, SCH], F32, tag='screp')
                nc.tensor.matmul(rep2[:, 0:512], sel[32:48, j, :], bc_t[32:48, t0:t0 + 512])
                nc.tensor.matmul(rep2[:, 512:SCH], sel[32:48, j, :], bc_t[32:48, t0 + 512:t1])
                hC = work.tile([128, SCH], BF16, tag='sc_hC')
                nc.vector.tensor_tensor(hC[:], h[:], rep2[:], op=ALU.mult)
                for sub in range(2):
                    nc.tensor.matmul(yps[sub][:], gmat[:],
                                     hC[:, sub * 512:(sub + 1) * 512],
                                     start=(j == 0), stop=(j == NJ - 1))
            tz2 = work.tile([64, SCH], BF16, tag='tl_tz2')
            nc.scalar.activation(tz2[:], zt[:, t0:t1], AF.Tanh, scale=0.5)
            sz2 = work.tile([64, SCH], BF16, tag='tl_sz2')
            nc.vector.scalar_tensor_tensor(sz2[:], tz2[:], 1.0, zt[:, t0:t1],
                                           op0=ALU.add, op1=ALU.mult)
            t2 = work.tile([64, SCH], BF16, tag='tl_t2')
            for sub in range(2):
                s0, s1 = sub * 512, (sub + 1) * 512
                yf = work.tile([64, 512], F32, tag='tl_yf')
                nc.vector.scalar_tensor_tensor(yf[:], scan_u[0:64, t0 + s0:t0 + s1],
                                               dph[:], yps[sub][:],
                                               op0=ALU.mult, op1=ALU.add)
                nc.vector.tensor_tensor(t2[:, s0:s1], yf[:], sz2[:, s0:s1],
                                        op=ALU.mult)
            if cross is None:
                ps = self.psum.tile([128, 8, 32], F32, tag='mm')
                for k in range(8):
                    nc.tensor.matmul(ps[:, k, :], t2[:, k * 128:(k + 1) * 128],
                                     wout[:])
                nc.scalar.copy(xT_out[:, c * 8:(c + 1) * 8, :], ps[:])
            else:
                for sub in range(2):
                    s0 = sub * 512
                    ps = self.psum.tile([32, 512], F32, tag='mm')
                    nc.tensor.matmul(ps[:], wout[:], t2[:, s0:s0 + 512])
                    nc.scalar.copy(gc[:, t0 + s0:t0 + s0 + 512], ps[:])
        return gc

    def single_block(self, pref, xT, resiT):
        nc = self.nc
        nc.vector.tensor_tensor(resiT[:], resiT[:], xT[:], op=ALU.add)
        xnT = self.work1.tile([128, 32, 32], F32, tag='xnT')
        self.ln_T(resiT, xnT)
        self.mamba(pref, xnT, xT)

    def cross_block(self, pref, msT, resiT, pan_nT):
        nc = self.nc
        fm, work = self.fm, self.work
        nc.vector.tensor_tensor(resiT[:], resiT[:], msT[:], op=ALU.add)
        n1w = self.bconst(pref + 'n1w', 'w_n1w')
        n1b = self.bconst(pref + 'n1b', 'w_n1b')
        ms1T = self.work1.tile([128, 32, 32], F32, tag='ms1T')
        self.ln_T(resiT, ms1T, affine=(n1w, n1b))
        xnT = self.work1.tile([128, 32, 32], F32, tag='xnT')
        self.ln_T(ms1T, xnT)
        # pan-side: x2 = 2*silu(dwconv2(ln(pan) @ win2 + b2))
        win2 = self.bconst(pref + 'win2', 'w_win2')
        win2b = self.bconst(pref + 'win2b', 'w_win2b')
        dwl2 = self.bconst(pref + 'dwl2', 'w_dwl2')
        dwb2 = self.bconst(pref + 'dwb2', 'w_dwb2')
        x2r = fm.tile([64, L], BF16, tag='m_x2r')
        self.mm_from_T(pan_nT, win2, x2r, bias=win2b[:], act='identity_bias')
        x2u = fm.tile([128, L], BF16, tag='m_x2u')
        for c in range(NT):
            t0, t1 = c * TCH, (c + 1) * TCH
            ps = self.psum.tile([128, TCH], F32, tag='mm')
            nc.tensor.matmul(ps[:], dwb2[:], self.onesrow[:], start=True, stop=False)
            for k in range(D_CONV):
                sh = 3 - k
                a0 = max(t0, sh)
                if a0 >= t1:
                    continue
                nc.tensor.matmul(ps[:, a0 - t0:TCH], dwl2[:, k, :],
                                 x2r[0:64, a0 - sh:t1 - sh],
                                 start=False, stop=(k == D_CONV - 1))
            th = work.tile([128, TCH], F32, tag='fe_th')
            nc.scalar.activation(th[:], ps[:], AF.Tanh, scale=0.5)
            nc.vector.scalar_tensor_tensor(x2u[:, t0:t1], th[:], 1.0, ps[:],
                                           op0=ALU.add, op1=ALU.mult)
        gc = self.mamba(pref, xnT, None, cross={'u2d': x2u})
        m = fm.tile([32, L], F32, tag='m_m')
        self.conv3x3([gc], [pref + 'dwsw'], pref + 'dwsb', 64, m, act='none',
                     resid=gc)
        self.transpose_to(m, msT)

    # ------------------------------------------------------------------
    def build(self):
        nc, work = self.nc, self.work
        persist = self.consts
        msT = persist.tile([128, 32, 32], F32, tag='p_msT')
        panT = persist.tile([128, 32, 32], F32, tag='p_panT')
        msR = persist.tile([128, 32, 32], F32, tag='p_msR')
        panR = persist.tile([128, 32, 32], F32, tag='p_panR')
        pan_nT = persist.tile([128, 32, 32], F32, tag='p_pannT')
        hs_bic_dram = self.dramp.tile([32, L], F32)

        with self.tc.tile_pool(name="encfm", bufs=1) as encfm:
            self.fm = encfm
            hs_bic = encfm.tile([32, L], F32, tag='eA')
            nc.vector.memset(hs_bic[:], 0.0)
            hs_in = self.dram['hs']  # (31, 256)
            hs_small = encfm.tile([31, 256], F32, tag='sm3')
            nc.sync.dma_start(hs_small[:], hs_in)
            hb = hs_bic[:]
            hsv = hs_small[:]
            for a in range(4):
                src = bass.AP(tensor=hsv.tensor, offset=hsv.offset,
                              ap=[[hsv.ap[0][0], 31], [16, 16], [1, 16], [0, 4]])
                dst = bass.AP(tensor=hb.tensor, offset=hb.offset + a * 64,
                              ap=[[hb.ap[0][0], 31], [256, 16], [1, 64]])
                nc.vector.tensor_copy(dst, src)
            nc.sync.dma_start(hs_bic_dram[:], hs_bic[:])
            pan = encfm.tile([32, L], F32, tag='eB')
            nc.vector.memset(pan[:], 0.0)
            nc.sync.dma_start(pan[0:1, :], self.dram['pan'])

            # dwt
            cat = encfm.tile([32, 1024], F32, tag='sm1')
            hf = encfm.tile([32, 1024], F32, tag='sm2')
            nc.vector.memset(cat[:], 0.0)
            nc.vector.memset(hf[:], 0.0)
            s_t = encfm.tile([31, 1024], F32, tag='eC')
            s2_t = encfm.tile([31, 1024], F32, tag='eD')
            d2_t = encfm.tile([31, 1024], F32, tag='eF')
            x1 = sview(hs_bic, 31, 64, 0, 0)
            x2 = sview(hs_bic, 31, 64, 1, 0)
            x3 = sview(hs_bic, 31, 64, 0, 1)
            x4 = sview(hs_bic, 31, 64, 1, 1)
            sv = s_t[:].rearrange("c (r w) -> c r w", r=32)
            s2v = s2_t[:].rearrange("c (r w) -> c r w", r=32)
            d2v = d2_t[:].rearrange("c (r w) -> c r w", r=32)
            nc.vector.tensor_tensor(sv, x1, x2, op=ALU.add)
            nc.vector.tensor_tensor(s2v, x3, x4, op=ALU.add)
            nc.vector.tensor_tensor(d2v, x3, x4, op=ALU.subtract)
            catv = cat[0:31, :].rearrange("c (r w) -> c r w", r=32)
            nc.vector.tensor_tensor(catv, sv, s2v, op=ALU.add)
            hfv = hf[0:31, :].rearrange("c (r w) -> c r w", r=32)
            nc.vector.tensor_tensor(hfv, s2v, sv, op=ALU.subtract)
            nc.vector.scalar_tensor_tensor(hfv, d2v, -2.0, hfv,
                                           op0=ALU.mult, op1=ALU.add)
            p1 = sview(pan, 1, 64, 0, 0)
            p2 = sview(pan, 1, 64, 1, 0)
            p3 = sview(pan, 1, 64, 0, 1)
            p4 = sview(pan, 1, 64, 1, 1)
            panll = encfm.tile([32, 1024], F32, tag='sm4')
            nc.vector.memset(panll[:], 0.0)
            plA = encfm.tile([1, 1024], F32, tag='sm5')
            plB = encfm.tile([1, 1024], F32, tag='sm6')
            pvA = plA[:].rearrange("c (r w) -> c r w", r=32)
            nc.vector.tensor_tensor(pvA, p1, p2, op=ALU.add)
            pvB = plB[:].rearrange("c (r w) -> c r w", r=32)
            nc.vector.tensor_tensor(pvB, p3, p4, op=ALU.add)
            nc.vector.tensor_tensor(panll[0:1, :], plA[:], plB[:], op=ALU.add)

            # encoders
            hs_f = self.encoder(hs_bic, 'he1', 64, 'eD', 'eC')   # -> eC
            pan_f = self.encoder(pan, 'pe1', 64, 'eD', 'eF')     # -> eF
            ll_f = self.encoder([cat, panll], 'hse', 32, 'sm3', 'sm1',
                                c0w=['hsec0wa', 'hsec0wb'])    # -> sm1
            hf_f = self.encoder(hf, 'pse', 32, 'sm4', 'sm2')     # -> sm2

            def up2_add(dst_t, src_t):
                d = dst_t[:]
                sv_ = src_t[:]
                for a in range(2):
                    dap = bass.AP(tensor=d.tensor, offset=d.offset + a * 64,
                                  ap=[[d.ap[0][0], 32], [128, 32], [2, 32], [1, 2]])
                    sap = bass.AP(tensor=sv_.tensor, offset=sv_.offset,
                                  ap=[[sv_.ap[0][0], 32], [32, 32], [1, 32], [0, 2]])
                    nc.vector.tensor_tensor(dap, dap, sap, op=ALU.add)
            up2_add(hs_f, ll_f)
            up2_add(pan_f, hf_f)

            hs_f2 = encfm.tile([32, L], BF16, tag='eD')
            self.conv3x3([hs_f, pan_f], ['sf1wa', 'sf1wb'], 'sf1bb', 64, hs_f2,
                         act='none', resid=hs_f)
            pan_f2 = encfm.tile([32, L], BF16, tag='eA')
            self.conv3x3([pan_f, hs_f2], ['sf2wa', 'sf2wb'], 'sf2bb', 64, pan_f2,
                         act='none', resid=pan_f)

            for (src, wn, bn, dst) in ((hs_f2, 'hstokw', 'hstokb', msT),
                                       (pan_f2, 'pantokw', 'pantokb', panT)):
                wt = self.gconst(wn)
                bt = self.gconst(bn)
                tok = encfm.tile([32, L], F32, tag='eB')
                for c in range(NT):
                    t0, t1 = c * TCH, (c + 1) * TCH
                    ps = self.psum.tile([32, TCH], F32, tag='mm')
                    nc.tensor.matmul(ps[:], wt[:], src[:, t0:t1])
                    nc.scalar.activation(tok[:, t0:t1], ps[:], AF.Identity,
                                         bias=bt[:])
                self.transpose_to(tok, dst)
            nc.vector.memset(msR[:], 0.0)
            nc.vector.memset(panR[:], 0.0)

        with self.tc.tile_pool(name="mamfm", bufs=1) as mamfm:
            self.fm = mamfm
            for i in range(8):
                self.single_block(f'ms{i}_', msT, msR)
            for i in range(8):
                self.single_block(f'pn{i}_', panT, panR)
            self.ln_T(panT, pan_nT)
            nc.vector.memset(msR[:], 0.0)
            for i in range(5):
                self.cross_block(f'df{i}_', msT, msR, pan_nT)

        with self.tc.tile_pool(name="rffm", bufs=1) as rffm:
            self.fm = rffm
            hs_out = rffm.tile([32, L], F32, tag='rA')
            self.transpose_back(msT, hs_out)
            out0 = rffm.tile([32, L], F32, tag='rB')
            self.conv3x3([hs_out], ['rfciw'], 'rfcib', 64, out0, act='none')
            r1 = rffm.tile([32, L], F32, tag='rC')
            self.conv3x3([out0], ['rfp1w'], 'rfp1b', 64, r1, act='relu')
            res = rffm.tile([32, L], F32, tag='rA')
            self.conv3x3([r1], ['rfp2w'], 'rfp2b', 64, res, act='none')
            ym = work.tile([32, 8], F32, tag='rf_ym')
            nc.vector.tensor_reduce(ym[:], res[:].rearrange("c (a b) -> c a b", a=8),
                                    axis=AX.X, op=ALU.add)
            ym1 = work.tile([32, 1], F32, tag='rf_ym1')
            nc.vector.tensor_reduce(ym1[:], ym[:], axis=AX.X, op=ALU.add)
            nc.scalar.mul(ym1[:], ym1[:], 1.0 / L)
            ps = self.psum.tile([8, 1], F32, tag='mm')
            nc.tensor.matmul(ps[:], self.gconst('rfd1w')[:], ym1[:])
            a1 = work.tile([8, 1], F32, tag='rf_a1')
            nc.scalar.activation(a1[:], ps[:], AF.Relu,
                                 bias=self.gconst('rfd1b')[:])
            ps2 = self.psum.tile([32, 1], F32, tag='mm')
            nc.tensor.matmul(ps2[:], self.gconst('rfd2w')[:], a1[:])
            zpre = work.tile([32, 1], F32, tag='rf_zpre')
            nc.scalar.activation(zpre[:], ps2[:], AF.Exp, scale=-1.0,
                                 bias=self.gconst('rfd2bneg')[:])
            zex = work.tile([32, 1], F32, tag='rf_zex')
            nc.vector.tensor_scalar_add(zex[:], zpre[:], 1.0)
            zatt = work.tile([32, 1], F32, tag='rf_zatt')
            nc.vector.reciprocal(zatt[:], zex[:])
            out1 = rffm.tile([32, L], F32, tag='rC')
            nc.vector.scalar_tensor_tensor(out1[:], res[:], zatt[:], out0[:],
                                           op0=ALU.mult, op1=ALU.add)
            hs_bic2 = rffm.tile([32, L], F32, tag='rA')
            nc.sync.dma_start(hs_bic2[:], hs_bic_dram[:])
            final = rffm.tile([32, L], F32, tag='rB')
            self.conv3x3([out1], ['rfclw'], 'rfclb', 64, final, act='none',
                         resid=hs_bic2, cout=31)
            nc.sync.dma_start(self.out_dram,
                              final[0:31, :].rearrange("c (h w) -> c h w", h=64))


# ============================================================================
# Host entry
# ============================================================================

_CACHE = {}


def _build_program(in_specs):
    key = tuple(sorted((k, tuple(s), str(d)) for k, (s, d) in in_specs.items()))
    if key in _CACHE:
        return _CACHE[key]
    from contextlib import ExitStack
    nc = bacc.Bacc("TRN2", target_bir_lowering=False, debug=False, num_devices=8)
    with tile.TileContext(nc) as tc:
        with ExitStack() as ctx:
            net = Net(nc, tc, ctx, in_specs)
            net.build()
    nc.compile()
    _CACHE[key] = nc
    return nc


def kernel(hs, pan, params):
    maps = prep_inputs(hs, pan, params)
    in_specs = {k: (v.shape, v.dtype) for k, v in maps[0].items()}
    nc = _build_program(in_specs)
    in_maps = []
    for core in range(8):
        m = maps[core % 2]
        in_maps.append({'in_' + k: v for k, v in m.items()})
    res = run_bass_kernel_spmd(nc, in_maps, core_ids=list(range(8)))
    out = np.stack([res.results[0]['out'], res.results[1]['out']], axis=0)
    return out.reshape(2, 31, 64, 64)
